# revision 19
# baseline (speedup 1.0000x reference)
"""TRN2 Bass kernel for nn_NeuralMemory (TTT-style fast-weight memory).

Math: per-token fast-weight tensors (blhd) are never materialized; the module
collapses to linear-attention-style L x L score-matrix matmuls:
    C   = wd_cs @ mom_cs                                   (combined decay)
    Zq1 = (C o (S+1)) @ gZ1s + wd_full * (q @ W1^T + b1),   S = q k^T
    Zq2 = (C o (T+1)) @ gZ2s + wd_full * (W2 Xq2 + b2),     T = Xq2 X2^T
with gZ1s/gZ2s the lr-scaled per-token grads and the decay matrices built from
exp-of-cumsum differences of log-sigmoid gates.

Perf design (vs. the 49us baseline):
  * every matmul operand is float32r -> single-pass PE matmuls (the baseline's
    fp32 LOW_HIGH double passes and all CAST instructions are gone)
  * ONE activation table load total: set 6 (natural_log_exp_and_others) is
    pre-placed manually; sigmoid/silu use exp + DVE reciprocal, not tanh
  * softplus = ln(1+exp(.)) in two small [128,3] ACTs
  * broadcasts (decay-row, WDF) are K=1 PE matmuls into PSUM, not gpsimd
  * biases enter via ACT bias columns / tensor-scalar columns / rank-1 matmuls
  * elementwise work is split across DVE and GpSimd
  * inputs arrive as packed dram params, sliced across 5 DMA queues
Sharding: data-parallel over batch (B=2): cores 0-3 batch 0, 4-7 batch 1.
"""
import sys
sys.path.insert(0, "/opt/trn_rl_repo")

import numpy as np
import concourse.bass as bass
from concourse import bacc
import concourse.mybir as mybir
import concourse.tile as tile
from concourse.bass_utils import run_bass_kernel_spmd

B, L, D, H = 2, 256, 128, 256
F32 = mybir.dt.float32
F32R = mybir.dt.float32r
AF = mybir.ActivationFunctionType
ALU = mybir.AluOpType

_CACHE = {}
LAST_RESULTS = None

# Wpack column layout (all f32r on device)
_WQKV = 0            # [d, 3*128]  WqT | WkT | WvT
_W1T = _WQKV + 384   # [d, h]
_W2DH = _W1T + 256   # [d, h]      W2_init as stored
_WEIGHTS_N = _W2DH + 256
_WSM = _WEIGHTS_N    # [d, 4]      [-Wm^T | -Wwd^T | Wlr^T | 0]
_TRI = _WSM + 4      # [m, l] upper-tri incl (m<=l)
_ONES = _TRI + 128   # [128, 128] ones
_IDT = _ONES + 128   # [128, 128] identity
_W2T = _IDT + 128    # [h%128, ht*128+d]
_WPACK_N = _W2T + 256

# bcols layout ([128, 7] f32): per-partition bias columns
(_BC_B1H0, _BC_B1H1, _BC_NB1H0, _BC_NB1H1, _BC_BQ, _BC_BK,
 _BC_BVF, _BC_B2) = range(8)

# brows layout ([128, 640] f32r, value/128 replicated on every partition):
# K=128 ones-matmul bias adds (keeps weight tile size uniform in acc groups)
_BR_B1 = 0           # [., 256] b1/128
_BR_SP = 384         # [., 3]  [-bm, -bwd, blr]/128


def _build():
    nc = bacc.Bacc("TRN2", target_bir_lowering=False, debug=False)

    xtd = nc.declare_dram_parameter("xT", [D, L], F32R, isOutput=False)
    wpd = nc.declare_dram_parameter("Wpack", [128, _WPACK_N], F32R, isOutput=False)
    bcd = nc.declare_dram_parameter("bcols", [128, 136], F32, isOutput=False)
    brd = nc.declare_dram_parameter("brows", [128, 640], F32R, isOutput=False)
    outd = nc.declare_dram_parameter("out", [L, D], F32, isOutput=True)

    with tile.TileContext(nc) as tc:
        with (
            tc.tile_pool(name="sb", bufs=1) as sb,
            tc.tile_pool(name="ps", bufs=4, space="PSUM") as ps,
            tc.tile_pool(name="pp", bufs=1, space="PSUM") as pp,
            tc.tile_pool(name="pss", bufs=2, space="PSUM") as pss,
        ):
            def mm_tile():
                return ps.tile([128, 256], F32, tag="mm", name="psmm")

            def sm_tile(shape, dt=F32):
                return pss.tile(shape, dt, tag="sm", name="pssm",
                                padded_shape=[128, 128])

            # ---- the single activation-table load (set 6: exp + ln) ----
            nc.scalar.add_instruction(mybir.InstLoadActFuncSet(
                name="I-actload6", act_func_set_id=6, ins=[], outs=[]))

            # ---- input DMAs, split across queues ----
            xT = sb.tile([128, 256], F32R, name="xT")
            nc.sync.dma_start(xT[:], xtd[:])
            brows = sb.tile([128, 640], F32R, name="brows")
            nc.sync.dma_start(brows[:], brd[:])

            statics = sb.tile([128, _WPACK_N - _WSM], F32R, name="statics")
            nc.scalar.dma_start(statics[:], wpd[:, _WSM:_WPACK_N])
            Wsm = statics[:, 0:4]
            tri = statics[:, _TRI - _WSM:_TRI - _WSM + 128]
            ones = statics[:, _ONES - _WSM:_ONES - _WSM + 128]
            identT = statics[:, _IDT - _WSM:_IDT - _WSM + 128]
            W2T = statics[:, _W2T - _WSM:_W2T - _WSM + 256]

            weights = sb.tile([128, _WEIGHTS_N], F32R, name="weights")
            nc.gpsimd.dma_start(weights[:], wpd[:, 0:_WEIGHTS_N])
            Wqkv = weights[:, _WQKV:_WQKV + 384]
            W1T = weights[:, _W1T:_W1T + 256]
            W2dh = weights[:, _W2DH:_W2DH + 256]
            bcols = sb.tile([128, 136], F32, name="bcols")
            nc.gpsimd.dma_start(bcols[:], bcd[:])
            identF = bcols[:, 8:136]

            ones_row = ones[0:1, 0:128]  # [1,128] f32r

            # decay matrices (compact: only nonzero blocks)
            # mom_cs: [0:128]=(m0,n0) tril, [128:256]=(m1,n0), [256:384]=(m1,n1) tril
            # wd_csT: [0:128]=(m0,l0) triu, [128:256]=(m0,l1), [256:384]=(m1,l1) triu
            mom_cs = sb.tile([128, 384], F32R, name="mom_cs")
            wd_csT = sb.tile([128, 384], F32R, name="wd_csT")

            # ================= gate chain =================
            # projections [l,4] = [-zm, -zwd, zlr, pad] per lt (free>=4 for
            # f32r matmul codegen), + K=128 bias add
            # sp8 cols: [sm0 sw0 sm1 sw1 lr0 lr1 junk junk]
            sp8 = sb.tile([128, 8], F32R, name="sp8")
            e6 = sb.tile([128, 6], F32, name="e6")
            for lt in range(2):
                p = sm_tile([128, 4])
                nc.tensor.matmul(p[:], xT[:, lt * 128:(lt + 1) * 128], Wsm,
                                 start=True, stop=False)
                nc.tensor.matmul(p[:], ones, brows[:, _BR_SP:_BR_SP + 4],
                                 start=False, stop=True, skip_group_check=True)
                sl = slice(lt * 3, (lt + 1) * 3)
                nc.scalar.activation(e6[:, sl], p[:, 0:3], AF.Exp)
                nc.scalar.activation(sp8[:, 2 * lt:2 * lt + 2],
                                     e6[:, 3 * lt:3 * lt + 2], AF.Ln, bias=1.0)
                nc.scalar.activation(sp8[:, 4 + lt:5 + lt],
                                     e6[:, 3 * lt + 2:3 * lt + 3], AF.Ln,
                                     bias=1.0)

            # cumsums: cs_ps cols = [cm0, cw0, cm1, cw1] (positive logs);
            # carry matmul targets cols 2:6 (4:6 are junk, never read)
            cs_ps = pss.tile([128, 8], F32, tag="sm", name="pssm",
                             padded_shape=[128, 128])
            nc.tensor.matmul(cs_ps[:, 0:4], tri, sp8[:, 0:4],
                             start=True, stop=False)
            nc.tensor.matmul(cs_ps[:, 2:6], ones, sp8[:, 0:4],
                             start=False, stop=True, skip_group_check=True)

            # columns to SBUF (full fp32: cumsums are offset-scale ~200 and
            # f32r's 11-bit mantissa would cost 5% after exp)
            cs_sb = sb.tile([128, 4], F32, name="cs_sb")
            nc.vector.tensor_copy(cs_sb[:], cs_ps[:, 0:4])
            cs_sb4 = cs_sb.rearrange("p (a b) -> p a b", a=2, b=2)
            negcm = sb.tile([128, 2], F32, name="negcm")
            nc.gpsimd.tensor_scalar(negcm[:], cs_sb4[:, :, 0:1], -1.0, None,
                                    ALU.mult)
            lrc = sb.tile([128, 2], F32, name="lrc")
            nc.gpsimd.tensor_scalar(lrc[:], sp8[:, 4:6], 1.0, None, ALU.mult)

            # rows via DMA-transpose (exact fp32; PE transpose would round
            # the ~200-magnitude cumsums to f32r)
            cm_row = sb.tile([1, 256], F32, name="cm_row")
            cw_row = sb.tile([1, 256], F32, name="cw_row")
            for lt in range(2):
                sl = slice(lt * 128, (lt + 1) * 128)
                rt = sm_tile([1, 128])
                nc.tensor.transpose(rt[:], cs_sb[:, 2 * lt:2 * lt + 1], identF)
                nc.vector.tensor_copy(cm_row[0:1, sl], rt[:])
                rt2 = sm_tile([1, 128])
                nc.tensor.transpose(rt2[:], cs_sb[:, 2 * lt + 1:2 * lt + 2],
                                    identF)
                nc.vector.tensor_copy(cw_row[0:1, sl], rt2[:])
            negcw_row = sb.tile([1, 256], F32, name="negcw_row")
            nc.vector.tensor_scalar(negcw_row[:], cw_row[:], -1.0, None,
                                    ALU.mult)
            wdf_row = sb.tile([1, 256], F32R, name="wdf_row")
            nc.scalar.activation(wdf_row[:], negcw_row[:], AF.Exp)

            # ---- decay eblocks: exp(bcast_row + bias_col), masked ----
            # mom_cs[m, n]: cols 0:128 (m0,n0, keep m>=n), 128:256 zero,
            #               256:384 (m1,n0 full), 384:512 (m1,n1, m>=n)
            bcm = sb.tile([128, 256], F32, name="bcm")
            nc.gpsimd.partition_broadcast(bcm[:], cm_row[:])
            nc.scalar.activation(mom_cs[:, 0:128], bcm[:, 0:128], AF.Exp,
                                 bias=negcm[:, 0:1])
            nc.scalar.activation(mom_cs[:, 128:384], bcm[:], AF.Exp,
                                 bias=negcm[:, 1:2])
            # wd_csT[m, l]: cols 0:128 (m0,l0, keep l>=m), 128:256 (m0,l1 full),
            #               256:384 (m1,l1, l>=m)
            bcw = sb.tile([128, 256], F32, name="bcw")
            nc.gpsimd.partition_broadcast(bcw[:], negcw_row[:])
            nc.scalar.activation(wd_csT[:, 0:256], bcw[:], AF.Exp,
                                 bias=cs_sb[:, 1:2])
            nc.scalar.activation(wd_csT[:, 256:384], bcw[:, 128:256], AF.Exp,
                                 bias=cs_sb[:, 3:4])
            # masks: keep m>=n (lower) on mom diag blocks, l>=m (upper) on wd
            for dst in (mom_cs[:, 0:128], mom_cs[:, 256:384]):
                nc.gpsimd.affine_select(out=dst, in_=dst, compare_op=ALU.is_ge,
                                        fill=0.0, base=0, pattern=[[-1, 128]],
                                        channel_multiplier=1)
            for dst in (wd_csT[:, 0:128], wd_csT[:, 256:384]):
                nc.gpsimd.affine_select(out=dst, in_=dst, compare_op=ALU.is_ge,
                                        fill=0.0, base=0, pattern=[[1, 128]],
                                        channel_multiplier=-1)

            # CT[n, l]: n0 needs both m-blocks; n1 only m1 ((m0,n1) is zero)
            ctb = pp.tile([128, 512], F32, tag="ct", name="ctb")
            ct0 = ctb[:, 0:256]
            nc.tensor.matmul(ct0, mom_cs[:, 0:128], wd_csT[:, 0:256],
                             start=True, stop=False)
            nc.tensor.matmul(ctb[:, 128:256], mom_cs[:, 128:256],
                             wd_csT[:, 256:384], start=False, stop=True,
                             skip_group_check=True)
            ct1 = ctb[:, 256:512]
            nc.tensor.matmul(ctb[:, 384:512], mom_cs[:, 256:384],
                             wd_csT[:, 256:384], start=True, stop=True,
                             skip_group_check=True)

            ct_sb = sb.tile([128, 512], F32, name="ct_sb")
            nc.gpsimd.memset(ct_sb[:, 256:384], 0.0)
            nc.vector.tensor_copy(ct_sb[:, 0:256], ct0)
            nc.vector.tensor_copy(ct_sb[:, 384:512], ctb[:, 384:512])
            cts_sb = (ct_sb[:, 0:256], ct_sb[:, 256:512])

            # WDF broadcast [128(part), l]
            auxb = pp.tile([128, 512], F32, tag="aux", name="auxb")
            wdfb = auxb[:, 0:256]
            nc.tensor.matmul(wdfb, ones_row, wdf_row[:], start=True, stop=True)
            wdfs = sb.tile([128, 256], F32, name="wdfs")
            nc.vector.tensor_copy(wdfs[:], wdfb)

            # ================= q/k/v =================
            kT = sb.tile([128, 256], F32R, name="kT")
            qT = sb.tile([128, 256], F32R, name="qT")
            kp = mm_tile()
            nc.tensor.matmul(kp[:], Wqkv[:, 128:256], xT[:], start=True, stop=True)
            nc.vector.tensor_scalar(kT[:], kp[:], bcols[:, _BC_BK:_BC_BK + 1],
                                    None, ALU.add)
            qp = mm_tile()
            nc.tensor.matmul(qp[:], Wqkv[:, 0:128], xT[:], start=True, stop=True)
            nc.vector.tensor_scalar(qT[:], qp[:], bcols[:, _BC_BQ:_BC_BQ + 1],
                                    None, ALU.add)
            vp = auxb[:, 256:512]
            nc.tensor.matmul(vp, Wqkv[:, 256:384], xT[:], start=True, stop=True,
                             skip_group_check=True)

            qTs = sb.tile([128, 256], F32R, name="qTs")
            nc.gpsimd.tensor_mul(qTs[:], qT[:], wdfs[:])

            # ================= layer-1 forward on k =================
            # [h, l] side: X2 = (Z1+b1) * sigmoid(Z1+b1)
            X2_hl = sb.tile([128, 512], F32R, name="X2_hl")
            for ht in range(2):
                p = mm_tile()
                nc.tensor.matmul(p[:], W1T[:, ht * 128:(ht + 1) * 128], kT[:],
                                 start=True, stop=True)
                eh = sb.tile([128, 256], F32, name=f"eh{ht}")
                nc.scalar.activation(
                    eh[:], p[:], AF.Exp, scale=-1.0,
                    bias=bcols[:, _BC_NB1H0 + ht:_BC_NB1H0 + ht + 1])
                nc.gpsimd.tensor_scalar(eh[:], eh[:], 1.0, None, ALU.add)
                sg = sb.tile([128, 256], F32, name=f"sgh{ht}")
                nc.vector.reciprocal(sg[:], eh[:])
                nc.vector.scalar_tensor_tensor(
                    X2_hl[:, ht * 256:(ht + 1) * 256], p[:],
                    bcols[:, _BC_B1H0 + ht:_BC_B1H0 + ht + 1], sg[:],
                    ALU.add, ALU.mult)

            # [l, h] side: silu'(z) = sig*(1 + z*(1-sig)), z = Z1+b1
            sbw = []
            for lt in range(2):
                p = mm_tile()
                nc.tensor.matmul(p[:], kT[:, lt * 128:(lt + 1) * 128], W1T[:],
                                 start=True, stop=False)
                nc.tensor.matmul(p[:], ones, brows[:, _BR_B1:_BR_B1 + 256],
                                 start=False, stop=True, skip_group_check=True)
                el = sb.tile([128, 256], F32, name=f"el{lt}")
                nc.scalar.activation(el[:], p[:], AF.Exp, scale=-1.0)
                nc.gpsimd.tensor_scalar(el[:], el[:], 1.0, None, ALU.add)
                sg = sb.tile([128, 256], F32, name=f"sgl{lt}")
                nc.vector.reciprocal(sg[:], el[:])
                t1 = sb.tile([128, 256], F32, name=f"t1l{lt}")
                nc.gpsimd.tensor_scalar(t1[:], sg[:], -1.0, 1.0, ALU.mult, ALU.add)
                u2 = sb.tile([128, 256], F32, name=f"u2l{lt}")
                nc.vector.tensor_mul(u2[:], p[:], t1[:])
                sbt = sb.tile([128, 256], F32, name=f"sb{lt}")
                nc.vector.scalar_tensor_tensor(sbt[:], u2[:], 1.0, sg[:],
                                               ALU.add, ALU.mult)
                sbw.append(sbt)

            # ================= layer-2 / grads =================
            z2 = mm_tile()
            for ht in range(2):
                nc.tensor.matmul(z2[:], W2T[:, ht * 128:(ht + 1) * 128],
                                 X2_hl[:, ht * 256:(ht + 1) * 256],
                                 start=(ht == 0), stop=(ht == 1))
            # gZ2 = Z2 + b2 - v - bv = z2 - (v + (bv - b2))
            vT2 = sb.tile([128, 256], F32, name="vT2")
            nc.vector.tensor_scalar(vT2[:], vp, bcols[:, _BC_BVF:_BC_BVF + 1],
                                    None, ALU.add)
            gZ2T = sb.tile([128, 256], F32R, name="gZ2T")
            nc.vector.tensor_sub(gZ2T[:], z2[:], vT2[:])

            # gZ2s[n, nt*128+d] = transpose(gZ2T) * lr
            gZ2s = sb.tile([128, 256], F32R, name="gZ2s")
            for nt in range(2):
                pt = sm_tile([128, 128], F32R)
                nc.tensor.transpose(pt[:], gZ2T[:, nt * 128:(nt + 1) * 128],
                                    identT)
                nc.vector.tensor_scalar(gZ2s[:, nt * 128:(nt + 1) * 128], pt[:],
                                        lrc[:, nt:nt + 1], None, ALU.mult)

            # gZ1s[n(lt), h] = (gX2 * lr) o silu'(Z1)
            gZ1s = sb.tile([128, 512], F32R, name="gZ1s")
            for lt in range(2):
                p = mm_tile()
                nc.tensor.matmul(p[:], gZ2T[:, lt * 128:(lt + 1) * 128], W2dh,
                                 start=True, stop=True)
                nc.vector.scalar_tensor_tensor(
                    gZ1s[:, lt * 256:(lt + 1) * 256], p[:], lrc[:, lt:lt + 1],
                    sbw[lt][:], ALU.mult, ALU.mult)

            # ================= scores / P1 =================
            P1T = sb.tile([128, 512], F32R, name="P1T")
            for nt in range(2):
                p = mm_tile()
                nc.tensor.matmul(p[:], kT[:, nt * 128:(nt + 1) * 128], qT[:],
                                 start=True, stop=True)
                nc.vector.scalar_tensor_tensor(
                    P1T[:, nt * 256:(nt + 1) * 256], p[:], 1.0, cts_sb[nt],
                    ALU.add, ALU.mult)

            # ================= query pass =================
            # Zq1[h, l] = P1-term + wdf*(W1 q + b1); Xq2 = silu(Zq1)
            Xq2T = sb.tile([128, 512], F32R, name="Xq2T")
            Xq2s = sb.tile([128, 512], F32R, name="Xq2s")
            for ht in range(2):
                p = mm_tile()
                for lt in range(2):
                    nc.tensor.matmul(
                        p[:],
                        gZ1s[:, lt * 256 + ht * 128:lt * 256 + (ht + 1) * 128],
                        P1T[:, lt * 256:(lt + 1) * 256],
                        start=(lt == 0), stop=False)
                nc.tensor.matmul(p[:], W1T[:, ht * 128:(ht + 1) * 128], qTs[:],
                                 start=False, stop=True)
                zq = sb.tile([128, 256], F32, name=f"zq{ht}")
                nc.vector.scalar_tensor_tensor(
                    zq[:], wdfs[:], bcols[:, _BC_B1H0 + ht:_BC_B1H0 + ht + 1],
                    p[:], ALU.mult, ALU.add)
                eq = sb.tile([128, 256], F32, name=f"eq{ht}")
                nc.scalar.activation(eq[:], zq[:], AF.Exp, scale=-1.0)
                nc.gpsimd.tensor_scalar(eq[:], eq[:], 1.0, None, ALU.add)
                sg = sb.tile([128, 256], F32, name=f"sgq{ht}")
                nc.vector.reciprocal(sg[:], eq[:])
                sl = slice(ht * 256, (ht + 1) * 256)
                nc.vector.tensor_mul(Xq2T[:, sl], zq[:], sg[:])
                nc.gpsimd.tensor_mul(Xq2s[:, sl], Xq2T[:, sl], wdfs[:])

            # T scores / P2
            P2T = sb.tile([128, 512], F32R, name="P2T")
            for nt in range(2):
                p = mm_tile()
                for ht in range(2):
                    nc.tensor.matmul(
                        p[:],
                        X2_hl[:, ht * 256 + nt * 128:ht * 256 + (nt + 1) * 128],
                        Xq2T[:, ht * 256:(ht + 1) * 256],
                        start=(ht == 0), stop=(ht == 1))
                nc.vector.scalar_tensor_tensor(
                    P2T[:, nt * 256:(nt + 1) * 256], p[:], 1.0, cts_sb[nt],
                    ALU.add, ALU.mult)

            # ================= output [d, l] -> transpose -> DMA ===========
            op = mm_tile()
            for nt in range(2):
                nc.tensor.matmul(op[:], gZ2s[:, nt * 128:(nt + 1) * 128],
                                 P2T[:, nt * 256:(nt + 1) * 256],
                                 start=(nt == 0), stop=False)
            for ht in range(2):
                nc.tensor.matmul(op[:], W2T[:, ht * 128:(ht + 1) * 128],
                                 Xq2s[:, ht * 256:(ht + 1) * 256],
                                 start=False, stop=(ht == 1))
            o_sb = sb.tile([128, 256], F32R, name="o_sb")
            nc.vector.scalar_tensor_tensor(o_sb[:], wdfs[:],
                                           bcols[:, _BC_B2:_BC_B2 + 1], op[:],
                                           ALU.mult, ALU.add)
            out_sb = sb.tile([128, 256], F32, name="out_sb")
            for lt in range(2):
                pt = sm_tile([128, 128], F32R)
                nc.tensor.transpose(pt[:], o_sb[:, lt * 128:(lt + 1) * 128],
                                    identT)
                sl = slice(lt * 128, (lt + 1) * 128)
                nc.scalar.copy(out_sb[:, sl], pt[:])
                nc.gpsimd.dma_start(outd[lt * 128:(lt + 1) * 128, :],
                                    out_sb[:, sl])

    nc.compile()
    n_loads = sum(isinstance(i, mybir.InstLoadActFuncSet)
                  for b in nc.main_func.blocks for i in b.instructions)
    assert n_loads <= 2, f"unexpected act table loads: {n_loads}"
    return nc


def kernel(**inputs):
    global LAST_RESULTS
    if "nc" not in _CACHE:
        _CACHE["nc"] = _build()
    nc = _CACHE["nc"]

    f = lambda a: np.ascontiguousarray(np.asarray(a, dtype=np.float32))
    wpack = np.zeros((128, _WPACK_N), dtype=np.float32)
    wpack[:, _WQKV:_WQKV + 128] = f(inputs["Wq"]).T
    wpack[:, _WQKV + 128:_WQKV + 256] = f(inputs["Wk"]).T
    wpack[:, _WQKV + 256:_WQKV + 384] = f(inputs["Wv"]).T
    wpack[:, _W1T:_W1T + 256] = f(inputs["W1_init"]).T
    wpack[:, _W2DH:_W2DH + 256] = f(inputs["W2_init"])
    w2t = f(inputs["W2_init"]).T  # [h, d]
    wpack[:, _W2T:_W2T + 128] = w2t[0:128]
    wpack[:, _W2T + 128:_W2T + 256] = w2t[128:256]
    wpack[:, _WSM + 0:_WSM + 1] = -f(inputs["Wm"]).T
    wpack[:, _WSM + 1:_WSM + 2] = -f(inputs["Wwd"]).T
    wpack[:, _WSM + 2:_WSM + 3] = f(inputs["Wlr"]).T
    wpack[:, _TRI:_TRI + 128] = np.triu(np.ones((128, 128), np.float32))
    wpack[:, _ONES:_ONES + 128] = 1.0
    wpack[:, _IDT:_IDT + 128] = np.eye(128, dtype=np.float32)

    b1 = f(inputs["b1_init"])
    bcols = np.zeros((128, 136), dtype=np.float32)
    bcols[:, _BC_B1H0] = b1[0:128]
    bcols[:, _BC_B1H1] = b1[128:256]
    bcols[:, _BC_NB1H0] = -b1[0:128]
    bcols[:, _BC_NB1H1] = -b1[128:256]
    bcols[:, _BC_BQ] = f(inputs["bq"])
    bcols[:, _BC_BK] = f(inputs["bk"])
    bcols[:, _BC_BVF] = f(inputs["bv"]) - f(inputs["b2_init"])
    bcols[:, _BC_B2] = f(inputs["b2_init"])
    bcols[:, 8:136] = np.eye(128, dtype=np.float32)

    brow1 = np.zeros((1, 640), dtype=np.float32)
    brow1[0, _BR_B1:_BR_B1 + 256] = b1
    brow1[0, _BR_SP + 0] = -float(np.asarray(inputs["bm"]).ravel()[0])
    brow1[0, _BR_SP + 1] = -float(np.asarray(inputs["bwd"]).ravel()[0])
    brow1[0, _BR_SP + 2] = float(np.asarray(inputs["blr"]).ravel()[0])
    brows = np.repeat(brow1 / 128.0, 128, axis=0)

    x = np.asarray(inputs["x"], dtype=np.float32)
    shared = {"Wpack": wpack, "bcols": bcols, "brows": brows}
    in_maps = []
    for core in range(8):
        m = dict(shared)
        m["xT"] = f(x[core // 4].T)
        in_maps.append(m)

    res = run_bass_kernel_spmd(nc, in_maps, core_ids=list(range(8)))
    LAST_RESULTS = res
    out = np.stack([res.results[0]["out"], res.results[4]["out"]], axis=0)
    return out.astype(np.float32)


# revision 22
# speedup vs baseline: 1.6794x; 1.6794x over previous
"""TRN2 Bass kernel for nn_NeuralMemory (TTT-style fast-weight memory).

Math: per-token fast-weight tensors (blhd) are never materialized; the module
collapses to linear-attention-style L x L score-matrix matmuls:
    C   = wd_cs @ mom_cs                                   (combined decay)
    Zq1 = (C o (S+1)) @ gZ1s + wd_full * (q @ W1^T + b1),   S = q k^T
    Zq2 = (C o (T+1)) @ gZ2s + wd_full * (W2 Xq2 + b2),     T = Xq2 X2^T
with gZ1s/gZ2s the lr-scaled per-token grads and the decay matrices built from
exp-of-cumsum differences of log-sigmoid gates.

Perf design (vs. the 49us baseline):
  * every matmul operand is float32r -> single-pass PE matmuls (the baseline's
    fp32 LOW_HIGH double passes and all CAST instructions are gone)
  * ONE activation table load total: set 6 (natural_log_exp_and_others) is
    pre-placed manually; sigmoid/silu use exp + DVE reciprocal, not tanh
  * softplus = ln(1+exp(.)) in two small [128,3] ACTs
  * broadcasts (decay-row, WDF) are K=1 PE matmuls into PSUM, not gpsimd
  * biases enter via ACT bias columns / tensor-scalar columns / rank-1 matmuls
  * elementwise work is split across DVE and GpSimd
  * inputs arrive as packed dram params, sliced across 5 DMA queues
Sharding: data-parallel over batch (B=2): cores 0-3 batch 0, 4-7 batch 1.
"""
import sys
sys.path.insert(0, "/opt/trn_rl_repo")

import numpy as np
import concourse.bass as bass
from concourse import bacc
import concourse.mybir as mybir
import concourse.tile as tile
from concourse.bass_utils import run_bass_kernel_spmd

B, L, D, H = 2, 256, 128, 256
F32 = mybir.dt.float32
F32R = mybir.dt.float32r
AF = mybir.ActivationFunctionType
ALU = mybir.AluOpType

_CACHE = {}
LAST_RESULTS = None

# Wpack column layout (all f32r on device)
_WQKV = 0            # [d, 3*128]  WqT | WkT | WvT
_W1T = _WQKV + 384   # [d, h]
_W2DH = _W1T + 256   # [d, h]      W2_init as stored
_WEIGHTS_N = _W2DH + 256
_WSM = _WEIGHTS_N    # [d, 4]      [-Wm^T | -Wwd^T | Wlr^T | 0]
_TRI = _WSM + 4      # [m, l] upper-tri incl (m<=l)
_ONES = _TRI + 128   # [128, 128] ones
_IDT = _ONES + 128   # [128, 128] identity
_W2T = _IDT + 128    # [h%128, ht*128+d]
_WPACK_N = _W2T + 256

# bcols layout ([128, 7] f32): per-partition bias columns
(_BC_B1H0, _BC_B1H1, _BC_NB1H0, _BC_NB1H1, _BC_BQ, _BC_BK,
 _BC_BVF, _BC_B2) = range(8)

# brows layout ([128, 640] f32r, value/128 replicated on every partition):
# K=128 ones-matmul bias adds (keeps weight tile size uniform in acc groups)
_BR_B1 = 0           # [., 256] b1/128
_BR_SP = 384         # [., 3]  [-bm, -bwd, blr]/128


def _build():
    nc = bacc.Bacc("TRN2", target_bir_lowering=False, debug=False)

    xtd = nc.declare_dram_parameter("xT", [D, L], F32R, isOutput=False)
    wpd = nc.declare_dram_parameter("Wpack", [128, _WPACK_N], F32R, isOutput=False)
    bcd = nc.declare_dram_parameter("bcols", [128, 136], F32, isOutput=False)
    brd = nc.declare_dram_parameter("brows", [128, 640], F32R, isOutput=False)
    outd = nc.declare_dram_parameter("out", [L, D], F32, isOutput=True)

    with tile.TileContext(nc) as tc:
        with (
            tc.tile_pool(name="sb", bufs=1) as sb,
            tc.tile_pool(name="ps", bufs=4, space="PSUM") as ps,
            tc.tile_pool(name="pp", bufs=1, space="PSUM") as pp,
            tc.tile_pool(name="pss", bufs=2, space="PSUM") as pss,
        ):
            def mm_tile():
                return ps.tile([128, 256], F32, tag="mm", name="psmm")

            def sm_tile(shape, dt=F32):
                return pss.tile(shape, dt, tag="sm", name="pssm",
                                padded_shape=[128, 128])

            # ---- the single activation-table load (set 6: exp + ln) ----
            nc.scalar.add_instruction(mybir.InstLoadActFuncSet(
                name="I-actload6", act_func_set_id=6, ins=[], outs=[]))

            # ---- input DMAs, split across queues ----
            xT = sb.tile([128, 256], F32R, name="xT")
            nc.sync.dma_start(xT[:], xtd[:])
            brows = sb.tile([128, 640], F32R, name="brows")
            nc.sync.dma_start(brows[:], brd[:])

            statics = sb.tile([128, _WPACK_N - _WSM], F32R, name="statics")
            nc.scalar.dma_start(statics[:], wpd[:, _WSM:_WPACK_N])
            Wsm = statics[:, 0:4]
            tri = statics[:, _TRI - _WSM:_TRI - _WSM + 128]
            ones = statics[:, _ONES - _WSM:_ONES - _WSM + 128]
            identT = statics[:, _IDT - _WSM:_IDT - _WSM + 128]
            W2T = statics[:, _W2T - _WSM:_W2T - _WSM + 256]

            weights = sb.tile([128, _WEIGHTS_N], F32R, name="weights")
            nc.gpsimd.dma_start(weights[:], wpd[:, 0:_WEIGHTS_N])
            Wqkv = weights[:, _WQKV:_WQKV + 384]
            W1T = weights[:, _W1T:_W1T + 256]
            W2dh = weights[:, _W2DH:_W2DH + 256]
            bcols = sb.tile([128, 136], F32, name="bcols")
            nc.gpsimd.dma_start(bcols[:], bcd[:])
            identF = bcols[:, 8:136]

            ones_row = ones[0:1, 0:128]  # [1,128] f32r

            # decay matrices (compact: only nonzero blocks)
            # mom_cs: [0:128]=(m0,n0) tril, [128:256]=(m1,n0), [256:384]=(m1,n1) tril
            # wd_csT: [0:128]=(m0,l0) triu, [128:256]=(m0,l1), [256:384]=(m1,l1) triu
            mom_cs = sb.tile([128, 384], F32R, name="mom_cs")
            wd_csT = sb.tile([128, 384], F32R, name="wd_csT")

            # ================= gate chain =================
            # projections [l,4] = [-zm, -zwd, zlr, pad] per lt (free>=4 for
            # f32r matmul codegen), + K=128 bias add
            # sp8 cols: [sm0 sw0 sm1 sw1 lr0 lr1 junk junk]
            sp8 = sb.tile([128, 8], F32R, name="sp8")
            e6 = sb.tile([128, 6], F32, name="e6")
            for lt in range(2):
                p = sm_tile([128, 4])
                nc.tensor.matmul(p[:], xT[:, lt * 128:(lt + 1) * 128], Wsm,
                                 start=True, stop=False)
                nc.tensor.matmul(p[:], ones, brows[:, _BR_SP:_BR_SP + 4],
                                 start=False, stop=True, skip_group_check=True)
                sl = slice(lt * 3, (lt + 1) * 3)
                nc.scalar.activation(e6[:, sl], p[:, 0:3], AF.Exp)
                nc.scalar.activation(sp8[:, 2 * lt:2 * lt + 2],
                                     e6[:, 3 * lt:3 * lt + 2], AF.Ln, bias=1.0)
                nc.scalar.activation(sp8[:, 4 + lt:5 + lt],
                                     e6[:, 3 * lt + 2:3 * lt + 3], AF.Ln,
                                     bias=1.0)

            # cumsums: cs_ps cols = [cm0, cw0, cm1, cw1] (positive logs);
            # carry matmul targets cols 2:6 (4:6 are junk, never read)
            cs_ps = pss.tile([128, 8], F32, tag="sm", name="pssm",
                             padded_shape=[128, 128])
            nc.tensor.matmul(cs_ps[:, 0:4], tri, sp8[:, 0:4],
                             start=True, stop=False)
            nc.tensor.matmul(cs_ps[:, 2:6], ones, sp8[:, 0:4],
                             start=False, stop=True, skip_group_check=True)

            # columns to SBUF (full fp32: cumsums are offset-scale ~200 and
            # f32r's 11-bit mantissa would cost 5% after exp)
            cs_sb = sb.tile([128, 4], F32, name="cs_sb")
            nc.vector.tensor_copy(cs_sb[:], cs_ps[:, 0:4])
            cs_sb4 = cs_sb.rearrange("p (a b) -> p a b", a=2, b=2)
            negcm = sb.tile([128, 2], F32, name="negcm")
            nc.gpsimd.tensor_scalar(negcm[:], cs_sb4[:, :, 0:1], -1.0, None,
                                    ALU.mult)
            lrc = sb.tile([128, 2], F32, name="lrc")
            nc.gpsimd.tensor_scalar(lrc[:], sp8[:, 4:6], 1.0, None, ALU.mult)

            # rows via DMA-transpose (exact fp32; PE transpose would round
            # the ~200-magnitude cumsums to f32r)
            cm_row = sb.tile([1, 256], F32, name="cm_row")
            cw_row = sb.tile([1, 256], F32, name="cw_row")
            for lt in range(2):
                sl = slice(lt * 128, (lt + 1) * 128)
                rt = sm_tile([1, 128])
                nc.tensor.transpose(rt[:], cs_sb[:, 2 * lt:2 * lt + 1], identF)
                nc.vector.tensor_copy(cm_row[0:1, sl], rt[:])
                rt2 = sm_tile([1, 128])
                nc.tensor.transpose(rt2[:], cs_sb[:, 2 * lt + 1:2 * lt + 2],
                                    identF)
                nc.vector.tensor_copy(cw_row[0:1, sl], rt2[:])
            negcw_row = sb.tile([1, 256], F32, name="negcw_row")
            nc.vector.tensor_scalar(negcw_row[:], cw_row[:], -1.0, None,
                                    ALU.mult)
            wdf_row = sb.tile([1, 256], F32R, name="wdf_row")
            nc.scalar.activation(wdf_row[:], negcw_row[:], AF.Exp)

            # ---- decay eblocks: exp(bcast_row + bias_col), masked ----
            # mom_cs[m, n]: cols 0:128 (m0,n0, keep m>=n), 128:256 zero,
            #               256:384 (m1,n0 full), 384:512 (m1,n1, m>=n)
            bcm = sb.tile([128, 256], F32, name="bcm")
            nc.gpsimd.partition_broadcast(bcm[:], cm_row[:])
            nc.scalar.activation(mom_cs[:, 0:128], bcm[:, 0:128], AF.Exp,
                                 bias=negcm[:, 0:1])
            nc.scalar.activation(mom_cs[:, 128:384], bcm[:], AF.Exp,
                                 bias=negcm[:, 1:2])
            # wd_csT[m, l]: cols 0:128 (m0,l0, keep l>=m), 128:256 (m0,l1 full),
            #               256:384 (m1,l1, l>=m)
            bcw = sb.tile([128, 256], F32, name="bcw")
            nc.gpsimd.partition_broadcast(bcw[:], negcw_row[:])
            nc.scalar.activation(wd_csT[:, 0:256], bcw[:], AF.Exp,
                                 bias=cs_sb[:, 1:2])
            nc.scalar.activation(wd_csT[:, 256:384], bcw[:, 128:256], AF.Exp,
                                 bias=cs_sb[:, 3:4])
            # masks: keep m>=n (lower) on mom diag blocks, l>=m (upper) on wd
            for dst in (mom_cs[:, 0:128], mom_cs[:, 256:384]):
                nc.gpsimd.affine_select(out=dst, in_=dst, compare_op=ALU.is_ge,
                                        fill=0.0, base=0, pattern=[[-1, 128]],
                                        channel_multiplier=1)
            for dst in (wd_csT[:, 0:128], wd_csT[:, 256:384]):
                nc.gpsimd.affine_select(out=dst, in_=dst, compare_op=ALU.is_ge,
                                        fill=0.0, base=0, pattern=[[1, 128]],
                                        channel_multiplier=-1)

            # CT[n, l]: n0 needs both m-blocks; n1 only m1 ((m0,n1) is zero)
            ctb = pp.tile([128, 512], F32, tag="ct", name="ctb")
            ct0 = ctb[:, 0:256]
            nc.tensor.matmul(ct0, mom_cs[:, 0:128], wd_csT[:, 0:256],
                             start=True, stop=False)
            nc.tensor.matmul(ctb[:, 128:256], mom_cs[:, 128:256],
                             wd_csT[:, 256:384], start=False, stop=True,
                             skip_group_check=True)
            ct1 = ctb[:, 256:512]
            nc.tensor.matmul(ctb[:, 384:512], mom_cs[:, 256:384],
                             wd_csT[:, 256:384], start=True, stop=True,
                             skip_group_check=True)

            # STTs cannot read two PSUM operands -> stage CT in SBUF.
            # ct_sb cols 256:384 = C[n1, l0] = 0 exactly.
            ct_sb = sb.tile([128, 512], F32, name="ct_sb")
            nc.gpsimd.memset(ct_sb[:, 256:384], 0.0)
            nc.vector.tensor_copy(ct_sb[:, 0:256], ct0)
            nc.vector.tensor_copy(ct_sb[:, 384:512], ctb[:, 384:512])
            cts_sb = (ct_sb[:, 0:256], ct_sb[:, 256:512])

            # WDF broadcast [128(part), l]
            auxb = pp.tile([128, 512], F32, tag="aux", name="auxb")
            wdfb = auxb[:, 0:256]
            nc.tensor.matmul(wdfb, ones_row, wdf_row[:], start=True, stop=True)
            wdfs = sb.tile([128, 256], F32, name="wdfs")
            nc.vector.tensor_copy(wdfs[:], wdfb)

            # ================= q/k/v =================
            kT = sb.tile([128, 256], F32R, name="kT")
            qT = sb.tile([128, 256], F32R, name="qT")
            kp = mm_tile()
            nc.tensor.matmul(kp[:], Wqkv[:, 128:256], xT[:], start=True, stop=True)
            nc.vector.tensor_scalar(kT[:], kp[:], bcols[:, _BC_BK:_BC_BK + 1],
                                    None, ALU.add)
            qp = mm_tile()
            nc.tensor.matmul(qp[:], Wqkv[:, 0:128], xT[:], start=True, stop=True)
            nc.vector.tensor_scalar(qT[:], qp[:], bcols[:, _BC_BQ:_BC_BQ + 1],
                                    None, ALU.add)
            vp = auxb[:, 256:512]
            nc.tensor.matmul(vp, Wqkv[:, 256:384], xT[:], start=True, stop=True,
                             skip_group_check=True)

            qTs = sb.tile([128, 256], F32R, name="qTs")
            nc.vector.tensor_mul(qTs[:], qT[:], wdfs[:])

            # ================= layer-1 forward on k =================
            # [h, l] side: X2 = (Z1+b1) * sigmoid(Z1+b1)
            X2_hl = sb.tile([128, 512], F32R, name="X2_hl")
            for ht in range(2):
                p = mm_tile()
                nc.tensor.matmul(p[:], W1T[:, ht * 128:(ht + 1) * 128], kT[:],
                                 start=True, stop=True)
                th = sb.tile([128, 256], F32, name=f"th{ht}")
                nc.scalar.activation(
                    th[:], p[:], AF.Tanh, scale=0.5,
                    bias=bcols[:, _BC_NB1H0 + ht:_BC_NB1H0 + ht + 1])
                sg = sb.tile([128, 256], F32, name=f"sgh{ht}")
                nc.vector.tensor_scalar(sg[:], th[:], 0.5, 0.5, ALU.mult,
                                        ALU.add)
                nc.vector.scalar_tensor_tensor(
                    X2_hl[:, ht * 256:(ht + 1) * 256], p[:],
                    bcols[:, _BC_B1H0 + ht:_BC_B1H0 + ht + 1], sg[:],
                    ALU.add, ALU.mult)

            # [l, h] side: silu'(z) = sig*(1 + z*(1-sig)), z = Z1+b1
            sbw = []
            for lt in range(2):
                p = mm_tile()
                nc.tensor.matmul(p[:], kT[:, lt * 128:(lt + 1) * 128], W1T[:],
                                 start=True, stop=False)
                nc.tensor.matmul(p[:], ones, brows[:, _BR_B1:_BR_B1 + 256],
                                 start=False, stop=True, skip_group_check=True)
                th = sb.tile([128, 256], F32, name=f"thl{lt}")
                nc.scalar.activation(th[:], p[:], AF.Tanh, scale=0.5)
                sg = sb.tile([128, 256], F32, name=f"sgl{lt}")
                nc.vector.tensor_scalar(sg[:], th[:], 0.5, 0.5, ALU.mult,
                                        ALU.add)
                t1 = sb.tile([128, 256], F32, name=f"t1l{lt}")
                nc.vector.tensor_scalar(t1[:], th[:], -0.5, 0.5, ALU.mult,
                                        ALU.add)
                u2 = sb.tile([128, 256], F32, name=f"u2l{lt}")
                nc.vector.tensor_mul(u2[:], p[:], t1[:])
                sbt = sb.tile([128, 256], F32, name=f"sb{lt}")
                nc.vector.scalar_tensor_tensor(sbt[:], u2[:], 1.0, sg[:],
                                               ALU.add, ALU.mult)
                sbw.append(sbt)

            # ================= layer-2 / grads =================
            z2 = mm_tile()
            for ht in range(2):
                nc.tensor.matmul(z2[:], W2T[:, ht * 128:(ht + 1) * 128],
                                 X2_hl[:, ht * 256:(ht + 1) * 256],
                                 start=(ht == 0), stop=(ht == 1))
            # gZ2 = Z2 + b2 - v - bv = z2 - (v + (bv-b2))
            vT2 = sb.tile([128, 256], F32, name="vT2")
            nc.vector.tensor_scalar(vT2[:], vp, bcols[:, _BC_BVF:_BC_BVF + 1],
                                    None, ALU.add)
            gZ2T = sb.tile([128, 256], F32R, name="gZ2T")
            nc.vector.tensor_sub(gZ2T[:], z2[:], vT2[:])

            # gZ2s[n, nt*128+d] = transpose(gZ2T) * lr
            gZ2s = sb.tile([128, 256], F32R, name="gZ2s")
            for nt in range(2):
                pt = sm_tile([128, 128], F32R)
                nc.tensor.transpose(pt[:], gZ2T[:, nt * 128:(nt + 1) * 128],
                                    identT)
                nc.vector.tensor_scalar(gZ2s[:, nt * 128:(nt + 1) * 128], pt[:],
                                        lrc[:, nt:nt + 1], None, ALU.mult)

            # gZ1s[n(lt), h] = (gX2 * lr) o silu'(Z1)
            gZ1s = sb.tile([128, 512], F32R, name="gZ1s")
            for lt in range(2):
                p = mm_tile()
                nc.tensor.matmul(p[:], gZ2T[:, lt * 128:(lt + 1) * 128], W2dh,
                                 start=True, stop=True)
                nc.vector.scalar_tensor_tensor(
                    gZ1s[:, lt * 256:(lt + 1) * 256], p[:], lrc[:, lt:lt + 1],
                    sbw[lt][:], ALU.mult, ALU.mult)

            # ================= scores / P1 =================
            P1T = sb.tile([128, 512], F32R, name="P1T")
            for nt in range(2):
                p = mm_tile()
                nc.tensor.matmul(p[:], kT[:, nt * 128:(nt + 1) * 128], qT[:],
                                 start=True, stop=True)
                nc.vector.scalar_tensor_tensor(
                    P1T[:, nt * 256:(nt + 1) * 256], p[:], 1.0, cts_sb[nt],
                    ALU.add, ALU.mult)

            # ================= query pass =================
            # Zq1[h, l] = P1-term + wdf*(W1 q + b1); Xq2 = silu(Zq1)
            Xq2T = sb.tile([128, 512], F32R, name="Xq2T")
            Xq2s = sb.tile([128, 512], F32R, name="Xq2s")
            for ht in range(2):
                p = mm_tile()
                for lt in range(2):
                    nc.tensor.matmul(
                        p[:],
                        gZ1s[:, lt * 256 + ht * 128:lt * 256 + (ht + 1) * 128],
                        P1T[:, lt * 256:(lt + 1) * 256],
                        start=(lt == 0), stop=False)
                nc.tensor.matmul(p[:], W1T[:, ht * 128:(ht + 1) * 128], qTs[:],
                                 start=False, stop=True)
                zq = sb.tile([128, 256], F32, name=f"zq{ht}")
                nc.vector.scalar_tensor_tensor(
                    zq[:], wdfs[:], bcols[:, _BC_B1H0 + ht:_BC_B1H0 + ht + 1],
                    p[:], ALU.mult, ALU.add)
                th = sb.tile([128, 256], F32, name=f"thq{ht}")
                nc.scalar.activation(th[:], zq[:], AF.Tanh, scale=0.5)
                sg = sb.tile([128, 256], F32, name=f"sgq{ht}")
                nc.vector.tensor_scalar(sg[:], th[:], 0.5, 0.5, ALU.mult,
                                        ALU.add)
                sl = slice(ht * 256, (ht + 1) * 256)
                nc.vector.tensor_mul(Xq2T[:, sl], zq[:], sg[:])
                nc.vector.tensor_mul(Xq2s[:, sl], Xq2T[:, sl], wdfs[:])

            # T scores / P2
            P2T = sb.tile([128, 512], F32R, name="P2T")
            for nt in range(2):
                p = mm_tile()
                for ht in range(2):
                    nc.tensor.matmul(
                        p[:],
                        X2_hl[:, ht * 256 + nt * 128:ht * 256 + (nt + 1) * 128],
                        Xq2T[:, ht * 256:(ht + 1) * 256],
                        start=(ht == 0), stop=(ht == 1))
                nc.vector.scalar_tensor_tensor(
                    P2T[:, nt * 256:(nt + 1) * 256], p[:], 1.0, cts_sb[nt],
                    ALU.add, ALU.mult)

            # ================= output [d, l] -> transpose -> DMA ===========
            op = mm_tile()
            for nt in range(2):
                nc.tensor.matmul(op[:], gZ2s[:, nt * 128:(nt + 1) * 128],
                                 P2T[:, nt * 256:(nt + 1) * 256],
                                 start=(nt == 0), stop=False)
            for ht in range(2):
                nc.tensor.matmul(op[:], W2T[:, ht * 128:(ht + 1) * 128],
                                 Xq2s[:, ht * 256:(ht + 1) * 256],
                                 start=False, stop=(ht == 1))
            o_sb = sb.tile([128, 256], F32R, name="o_sb")
            nc.vector.scalar_tensor_tensor(o_sb[:], wdfs[:],
                                           bcols[:, _BC_B2:_BC_B2 + 1], op[:],
                                           ALU.mult, ALU.add)
            out_sb = sb.tile([128, 256], F32, name="out_sb")
            for lt in range(2):
                pt = sm_tile([128, 128], F32R)
                nc.tensor.transpose(pt[:], o_sb[:, lt * 128:(lt + 1) * 128],
                                    identT)
                sl = slice(lt * 128, (lt + 1) * 128)
                nc.scalar.copy(out_sb[:, sl], pt[:])
                nc.gpsimd.dma_start(outd[lt * 128:(lt + 1) * 128, :],
                                    out_sb[:, sl])

    nc.compile()
    n_loads = sum(isinstance(i, mybir.InstLoadActFuncSet)
                  for b in nc.main_func.blocks for i in b.instructions)
    assert n_loads <= 2, f"unexpected act table loads: {n_loads}"
    return nc


def kernel(**inputs):
    global LAST_RESULTS
    if "nc" not in _CACHE:
        _CACHE["nc"] = _build()
    nc = _CACHE["nc"]

    f = lambda a: np.ascontiguousarray(np.asarray(a, dtype=np.float32))
    wpack = np.zeros((128, _WPACK_N), dtype=np.float32)
    wpack[:, _WQKV:_WQKV + 128] = f(inputs["Wq"]).T
    wpack[:, _WQKV + 128:_WQKV + 256] = f(inputs["Wk"]).T
    wpack[:, _WQKV + 256:_WQKV + 384] = f(inputs["Wv"]).T
    wpack[:, _W1T:_W1T + 256] = f(inputs["W1_init"]).T
    wpack[:, _W2DH:_W2DH + 256] = f(inputs["W2_init"])
    w2t = f(inputs["W2_init"]).T  # [h, d]
    wpack[:, _W2T:_W2T + 128] = w2t[0:128]
    wpack[:, _W2T + 128:_W2T + 256] = w2t[128:256]
    wpack[:, _WSM + 0:_WSM + 1] = -f(inputs["Wm"]).T
    wpack[:, _WSM + 1:_WSM + 2] = -f(inputs["Wwd"]).T
    wpack[:, _WSM + 2:_WSM + 3] = f(inputs["Wlr"]).T
    wpack[:, _TRI:_TRI + 128] = np.triu(np.ones((128, 128), np.float32))
    wpack[:, _ONES:_ONES + 128] = 1.0
    wpack[:, _IDT:_IDT + 128] = np.eye(128, dtype=np.float32)

    b1 = f(inputs["b1_init"])
    bcols = np.zeros((128, 136), dtype=np.float32)
    bcols[:, _BC_B1H0] = b1[0:128]
    bcols[:, _BC_B1H1] = b1[128:256]
    bcols[:, _BC_NB1H0] = -b1[0:128]
    bcols[:, _BC_NB1H1] = -b1[128:256]
    bcols[:, _BC_BQ] = f(inputs["bq"])
    bcols[:, _BC_BK] = f(inputs["bk"])
    bcols[:, _BC_BVF] = f(inputs["bv"]) - f(inputs["b2_init"])
    bcols[:, _BC_B2] = f(inputs["b2_init"])
    bcols[:, 8:136] = np.eye(128, dtype=np.float32)

    brow1 = np.zeros((1, 640), dtype=np.float32)
    brow1[0, _BR_B1:_BR_B1 + 256] = b1
    brow1[0, _BR_SP + 0] = -float(np.asarray(inputs["bm"]).ravel()[0])
    brow1[0, _BR_SP + 1] = -float(np.asarray(inputs["bwd"]).ravel()[0])
    brow1[0, _BR_SP + 2] = float(np.asarray(inputs["blr"]).ravel()[0])
    brows = np.repeat(brow1 / 128.0, 128, axis=0)

    x = np.asarray(inputs["x"], dtype=np.float32)
    shared = {"Wpack": wpack, "bcols": bcols, "brows": brows}
    in_maps = []
    for core in range(8):
        m = dict(shared)
        m["xT"] = f(x[core // 4].T)
        in_maps.append(m)

    res = run_bass_kernel_spmd(nc, in_maps, core_ids=list(range(8)))
    LAST_RESULTS = res
    out = np.stack([res.results[0]["out"], res.results[4]["out"]], axis=0)
    return out.astype(np.float32)
